# revision 15
# baseline (speedup 1.0000x reference)
"""Trainium2 Bass kernel for nn_AttModel (masked-attention GNN message passing).

Computation (per batch b):
    q/k/v = relu(x @ W*^T + b*)            [N, H]
    S     = q @ k^T                        [N, N]
    att   = softmax(S*mask - NEG*(1-mask)) [N, N]
    y     = relu((att @ v) @ Wo^T + bo)    [N, DOUT]

Sharding: data-parallel over (batch, query-half) -> 8 cores, each owning
2048 query rows of one batch. Zero cross-core communication.

Key identity: exp(S*m - NEG*(1-m)) == m * exp(S) elementwise for m in {0,1}
(exp(S) <= e^33 is finite in bf16/f32), so the kernel computes
    P^T = maskT (*) exp(S^T)
with exp reading the f32 PSUM scores directly on ACT (the critical engine:
64 x [128,1024] exp instructions ~= 66us) and the mask applied afterwards
as an all-bf16 SBUF multiply on DVE at 2x throughput. The softmax
denominator Z is built from two DVE bf16 partial-sum adds (4 key-chunks ->
1) followed by one cheap rank-1 PE ones-matmul per macro-step.

Layout-only host prep (same class as the baseline's mask transpose): x, W
are sent pre-transposed in bf16 so the kernel needs no on-chip transposes
or casts for them; the mask is sent transposed ([key, query]) in bf16.

Per-core structure:
  - setup: q/k/v projections from xT (PE) with relu+bias epilogues on
    ACT (early groups, hiding in ACT's idle head) and DVE (tensor_scalar
    fused add-bias+max). v is computed in natural [key, h] layout directly
    (lhsT = xT chunk), with its free-axis bias pre-seeded into PSUM by a
    rank-1 PE matmul so the epilogue is a plain max(.,0) copy.
  - main loop: 32 macro-steps (MS) of 4 key-chunks x 512 queries:
    4 S-matmuls -> 2 exps -> mask-mult -> 2 Z-presum adds -> 4 AV matmuls
    + 1 Z ones-matmul. Per-MS engine busy ~= ACT 2076ns / DVE 2048ns /
    PE 1917ns / DMA 1456ns. A few MS per group run the mult/presums on
    gpsimd (otherwise idle) with their AV/Z matmuls deferred to the end of
    the group so the in-order PE queue never waits on slow gpsimd ops.
  - epilogue per 512-query group: Z -> query partitions via k=1 transpose
    matmuls, G = U^T Wo in bf16, y = relu(G/Z + bo) via DVE STT + gpsimd
    relu.

Environment note: this walrus build rejects instructions with more than
one semaphore wait; _split_multiwaits rewrites the Tile output to
single-wait form (extra waits move to preceding NoOps, same semantics).
"""
import ml_dtypes
import numpy as np

import concourse.bass as bass
import concourse.mybir as mybir
from concourse.tile import TileContext
from concourse.bass_utils import run_bass_kernel_spmd

B, N, DIN, H, DOUT = 4, 4096, 128, 128, 128
NQ = N // 2  # 2048 query rows per core
CORES = 8

F32 = mybir.dt.float32
BF16 = mybir.dt.bfloat16
AL = mybir.AluOpType
AF = mybir.ActivationFunctionType

_wfix_ctr = [0]


def _split_multiwaits(nc):
    """This walrus build rejects instructions carrying >1 semaphore wait.

    Move all but the last wait of each instruction onto preceding NoOps on
    the same engine (engine streams execute in order, so semantics hold).
    """
    for func in nc.m.functions:
        for block in func.blocks:
            new_insts = []
            changed = False
            for inst in block.instructions:
                si = inst.sync_info
                waits = list(si.on_wait) if si is not None else []
                if len(waits) > 1:
                    for w in waits[:-1]:
                        _wfix_ctr[0] += 1
                        nop = mybir.InstNoOp(
                            name=f"WFIX-{_wfix_ctr[0]}", ins=[], outs=[]
                        )
                        nop.engine = inst.engine
                        nop.sync_info = mybir.SyncInfo(on_wait=[w], on_update=[])
                        new_insts.append(nop)
                    si.on_wait = [waits[-1]]
                    inst.sync_info = si
                    changed = True
                new_insts.append(inst)
            if changed:
                block.instructions = new_insts


def _build_nc(gps_jj=(0, 4), act_relu_groups=4):
    """gps_jj: which macro-steps (jj index) of each group run the
    mask-mult + Z-presum chain on gpsimd instead of DVE (their AV/Z
    matmuls are deferred to group end). act_relu_groups: how many of the
    earliest projection relus run on ACT (they hide in ACT's idle head);
    the rest run on DVE."""
    nc = bass.Bass()

    xTq_d = nc.dram_tensor("xTq", [DIN, NQ], BF16, kind="ExternalInput")
    xTkv_d = nc.dram_tensor("xTkv", [DIN, N], BF16, kind="ExternalInput")
    maskT_d = nc.dram_tensor("maskT", [N, NQ], BF16, kind="ExternalInput")
    wqT_d = nc.dram_tensor("WqT", [DIN, H], BF16, kind="ExternalInput")
    wkT_d = nc.dram_tensor("WkT", [DIN, H], BF16, kind="ExternalInput")
    wvT_d = nc.dram_tensor("WvT", [DIN, H], BF16, kind="ExternalInput")
    woT_d = nc.dram_tensor("WoT", [H, DOUT], BF16, kind="ExternalInput")
    bq_d = nc.dram_tensor("bq", [H], F32, kind="ExternalInput")
    bk_d = nc.dram_tensor("bk", [H], F32, kind="ExternalInput")
    bv4_d = nc.dram_tensor("bv4", [4 * H], BF16, kind="ExternalInput")
    bo_d = nc.dram_tensor("bo", [DOUT], F32, kind="ExternalInput")
    y_d = nc.dram_tensor("y", [NQ, DOUT], F32, kind="ExternalOutput")

    with TileContext(nc) as tc:
        with tc.tile_pool(name="singles", bufs=1) as singles:
            ones16 = singles.tile([128, 1], BF16, tag="ones16")
            nc.vector.memset(ones16, 1.0)
            onesrow = singles.tile([1, 128], BF16, tag="onesrow")
            nc.vector.memset(onesrow, 1.0)
            one_f32 = singles.tile([1, 1], F32, tag="one_f32")
            nc.vector.memset(one_f32, 1.0)

            # Setup-only DMAs go on the ACT queue (idle early) so the x/mask
            # stream on the SP queue is not delayed by DGE serialization.
            bq_sb = singles.tile([128, 1], F32, tag="bq")
            bk_sb = singles.tile([128, 1], F32, tag="bk")
            bo_sb = singles.tile([128, 128], F32, tag="bo")
            bv4_sb = singles.tile([1, 512], BF16, tag="bv4")
            wqT = singles.tile([128, 128], BF16, tag="wqT")
            wkT = singles.tile([128, 128], BF16, tag="wkT")
            wvT = singles.tile([128, 128], BF16, tag="wvT")
            woT = singles.tile([128, 128], BF16, tag="woT")
            nc.scalar.dma_start(out=wqT, in_=wqT_d[:])
            nc.scalar.dma_start(
                out=bq_sb, in_=bq_d[:].rearrange("(p o) -> p o", o=1)
            )
            nc.scalar.dma_start(out=wkT, in_=wkT_d[:])
            nc.scalar.dma_start(
                out=bk_sb, in_=bk_d[:].rearrange("(p o) -> p o", o=1)
            )
            nc.scalar.dma_start(out=wvT, in_=wvT_d[:])
            nc.scalar.dma_start(
                out=bv4_sb, in_=bv4_d[:].rearrange("(o n) -> o n", o=1)
            )
            nc.scalar.dma_start(out=woT, in_=woT_d[:])
            nc.scalar.dma_start(
                out=bo_sb,
                in_=bass.AP(tensor=bo_d, offset=0, ap=[[0, 128], [1, 128]]),
            )

            xTq = singles.tile([128, 4, 512], BF16, tag="xTq")
            xTkv = singles.tile([128, 8, 512], BF16, tag="xTkv")

            qT = [
                singles.tile([128, 512], BF16, tag=f"qT{g}", name=f"qT{g}")
                for g in range(4)
            ]
            kT = [
                singles.tile([128, 512], BF16, tag=f"kT{g}", name=f"kT{g}")
                for g in range(8)
            ]
            vn = [
                singles.tile([128, 4, 128], BF16, tag=f"vn{g}", name=f"vn{g}")
                for g in range(8)
            ]

            with tc.tile_pool(name="setup_ps", bufs=3, space="PSUM") as sps:
                # Interleave projections so group-0 dependencies clear first.
                # Each step: one x-slice DMA already issued; matmul -> relu.
                def q_proj(g, on_act):
                    nc.sync.dma_start(
                        out=xTq[:, g, :], in_=xTq_d[:, g * 512:(g + 1) * 512]
                    )
                    pq = sps.tile([128, 512], F32, tag="proj", name=f"pq{g}")
                    nc.tensor.matmul(
                        out=pq, lhsT=wqT, rhs=xTq[:, g, :],
                        start=True, stop=True,
                    )
                    if on_act:
                        nc.scalar.activation(
                            out=qT[g][:], in_=pq, func=AF.Relu, bias=bq_sb
                        )
                    else:
                        nc.vector.tensor_scalar(
                            out=qT[g][:], in0=pq, scalar1=bq_sb, scalar2=0.0,
                            op0=AL.add, op1=AL.max,
                        )

                def k_proj(g, on_act):
                    nc.sync.dma_start(
                        out=xTkv[:, g, :], in_=xTkv_d[:, g * 512:(g + 1) * 512]
                    )
                    pk = sps.tile([128, 512], F32, tag="proj", name=f"pk{g}")
                    nc.tensor.matmul(
                        out=pk, lhsT=wkT, rhs=xTkv[:, g, :],
                        start=True, stop=True,
                    )
                    if on_act:
                        nc.scalar.activation(
                            out=kT[g][:], in_=pk, func=AF.Relu, bias=bk_sb
                        )
                    else:
                        nc.vector.tensor_scalar(
                            out=kT[g][:], in0=pk, scalar1=bk_sb, scalar2=0.0,
                            op0=AL.add, op1=AL.max,
                        )

                def v_proj(g):
                    # v natural [key, h]: per chunk lhsT = xT chunk, with the
                    # free-axis bias pre-seeded into PSUM by a rank-1 matmul.
                    pv = sps.tile([128, 4, 128], F32, tag="proj",
                                  name=f"pv{g}")
                    nc.tensor.matmul(
                        out=pv.rearrange("p a b -> p (a b)"),
                        lhsT=onesrow, rhs=bv4_sb,
                        start=True, stop=False, skip_group_check=True,
                    )
                    for c in range(4):
                        nc.tensor.matmul(
                            out=pv[:, c, :],
                            lhsT=xTkv[:, g, c * 128:(c + 1) * 128],
                            rhs=wvT,
                            start=False, stop=True, skip_group_check=True,
                        )
                    nc.vector.tensor_scalar(
                        out=vn[g][:].rearrange("p a b -> p (a b)"),
                        in0=pv.rearrange("p a b -> p (a b)"),
                        scalar1=0.0, scalar2=None, op0=AL.max,
                    )

                # v_proj(g) consumes xTkv chunks 4g..4g+3 = slices g//2, but
                # k_proj(g) loads slice g; emit v after the k that loads it.
                relu_budget = [act_relu_groups]

                def on_act():
                    relu_budget[0] -= 1
                    return relu_budget[0] >= 0

                q_proj(0, on_act())
                k_proj(0, on_act())
                k_proj(1, on_act())
                v_proj(0)
                q_proj(1, on_act())
                k_proj(2, on_act())
                k_proj(3, on_act())
                v_proj(1)
                q_proj(2, on_act())
                k_proj(4, on_act())
                k_proj(5, on_act())
                v_proj(2)
                q_proj(3, on_act())
                k_proj(6, on_act())
                k_proj(7, on_act())
                v_proj(3)
                for g in range(4, 8):
                    v_proj(g)

            _main(nc, tc, maskT_d, y_d, qT, kT, vn, woT, bo_sb,
                  ones16, one_f32, gps_jj)

    _split_multiwaits(nc)
    return nc


def _main(nc, tc, maskT_d, y_d, qT, kT, vn, woT, bo_sb, ones16, one_f32,
          gps_jj):
    with (
        tc.tile_pool(name="mqp", bufs=4) as mqpool,
        tc.tile_pool(name="ep", bufs=6) as epool,
        tc.tile_pool(name="zap", bufs=3) as zapool,
        tc.tile_pool(name="zbp", bufs=6) as zbpool,
        tc.tile_pool(name="tiny", bufs=8) as tinypool,
        tc.tile_pool(name="utsb", bufs=2) as utsbp,
        tc.tile_pool(name="outb", bufs=8) as ypool,
        tc.tile_pool(name="sps2", bufs=2, space="PSUM") as spsum,
        tc.tile_pool(name="utps", bufs=2, space="PSUM") as utpsum,
        tc.tile_pool(name="zps", bufs=1, space="PSUM") as zpsum,
    ):
        pending = [None]

        def emit_ms(ig, jj, utp, zp, state):
            """One macro-step: 4 key-chunks x 512 queries."""
            on_gps = jj in gps_jj
            mq = mqpool.tile([128, 4, 512], BF16, tag="mq",
                             name=f"mq{ig}_{jj}")
            nc.sync.dma_start(
                out=mq,
                in_=maskT_d[
                    jj * 512:(jj + 1) * 512,
                    ig * 512:(ig + 1) * 512,
                ].rearrange("(c p) i -> p c i", p=128),
            )
            E = epool.tile([128, 2048], BF16, tag="E", name=f"E{ig}_{jj}")
            for half in range(2):
                sp = spsum.tile([128, 2, 512], F32, tag="s",
                                name=f"sp{ig}_{jj}_{half}")
                for w in range(2):
                    jc = 4 * jj + 2 * half + w
                    nc.tensor.matmul(
                        out=sp[:, w, :],
                        lhsT=kT[jc // 4][:, (jc % 4) * 128:(jc % 4 + 1) * 128],
                        rhs=qT[ig][:],
                        start=True, stop=True,
                    )
                nc.scalar.activation(
                    out=E[:, half * 1024:(half + 1) * 1024],
                    in_=sp.rearrange("p a b -> p (a b)"),
                    func=AF.Exp,
                )
            eng = nc.gpsimd if on_gps else nc.vector
            # P^T = maskT * exp(S^T), all-bf16 (2x on DVE)
            eng.tensor_tensor(
                out=E[:], in0=E[:],
                in1=mq[:].rearrange("p a b -> p (a b)"), op=AL.mult,
            )
            # Z partial sums: 4 chunks -> 1 [128, 512] tile
            za = zapool.tile([128, 1024], BF16, tag="za",
                             name=f"za{ig}_{jj}")
            eng.tensor_tensor(
                out=za, in0=E[:, 0:1024], in1=E[:, 1024:2048], op=AL.add
            )
            zb = zbpool.tile([128, 512], BF16, tag="zb", name=f"zb{ig}_{jj}")
            eng.tensor_tensor(
                out=zb, in0=za[:, 0:512], in1=za[:, 512:1024], op=AL.add
            )

            def av_z(first, last):
                for w in range(4):
                    g4, c4 = divmod(4 * jj + w, 4)
                    nc.tensor.matmul(
                        out=utp,
                        lhsT=vn[g4][:, c4, :],
                        rhs=E[:, w * 512:(w + 1) * 512],
                        start=(first and w == 0),
                        stop=(last and w == 3),
                    )
                nc.tensor.matmul(
                    out=zp, lhsT=ones16, rhs=zb, start=first, stop=last,
                )

            if on_gps:
                state["deferred"].append(av_z)
            else:
                first = state["n_inline"] == 0
                state["n_inline"] += 1
                last = not gps_jj and state["n_inline"] == 8
                av_z(first, last)

        def make_epilogue_tail(ig, zrow, uts):
            def tail():
                ztp = zpsum.tile([128, 4], F32, tag="epi", name=f"ztp{ig}",
                                 padded_shape=[128, 128])
                for i4 in range(4):
                    nc.tensor.matmul(
                        out=ztp[:, i4:i4 + 1],
                        lhsT=zrow[:, i4 * 128:(i4 + 1) * 128],
                        rhs=one_f32,
                        start=True, stop=True,
                    )
                rz4 = tinypool.tile([128, 4], F32, tag="rz4",
                                    name=f"rz4_{ig}")
                nc.vector.reciprocal(rz4, ztp)
                yt = ypool.tile([128, 4, 128], F32, tag="y", name=f"y{ig}")
                for ib in range(4):
                    g = zpsum.tile([128, 128], F32, tag="epi",
                                   name=f"g{ig}_{ib}")
                    nc.tensor.matmul(
                        out=g, lhsT=uts[:, ib * 128:(ib + 1) * 128], rhs=woT,
                        start=True, stop=True,
                    )
                    u = ypool.tile([128, 128], F32, tag="u",
                                   name=f"u{ig}_{ib}")
                    nc.vector.scalar_tensor_tensor(
                        out=u, in0=g, scalar=rz4[:, ib:ib + 1],
                        op0=AL.mult, in1=bo_sb, op1=AL.add,
                    )
                    nc.gpsimd.tensor_scalar(
                        out=yt[:, ib, :], in0=u, scalar1=0.0, scalar2=None,
                        op0=AL.max,
                    )
                nc.sync.dma_start(
                    out=y_d[ig * 512:(ig + 1) * 512, :].rearrange(
                        "(c p) o -> p c o", p=128
                    ),
                    in_=yt,
                )
            return tail

        for ig in range(4):
            utp = utpsum.tile([128, 512], F32, tag="ut", name=f"utp{ig}")
            zp = zpsum.tile([1, 512], F32, tag="z", name=f"zp{ig}")
            state = {"deferred": [], "n_inline": 0}
            for jj in range(8):
                emit_ms(ig, jj, utp, zp, state)
                if jj == 2 and pending[0] is not None:
                    # previous group's PE-light epilogue tail, emitted here so
                    # the new group's S-matmuls precede it in the PE queue
                    pending[0]()
                    pending[0] = None
            for i, av_z in enumerate(state["deferred"]):
                av_z(False, i == len(state["deferred"]) - 1)
            # drain the PSUM accumulators immediately (frees zp/utp slots)
            zrow = tinypool.tile([1, 512], F32, tag="zrow", name=f"zr{ig}")
            nc.vector.tensor_copy(zrow, zp)
            uts = utsbp.tile([128, 512], BF16, tag="uts", name=f"uts{ig}")
            nc.vector.tensor_copy(uts, utp)
            pending[0] = make_epilogue_tail(ig, zrow, uts)
        pending[0]()


_NC_CACHE = {}


def _get_nc(gps_jj=(0, 4), act_relu_groups=4):
    key = (tuple(gps_jj), act_relu_groups)
    if key not in _NC_CACHE:
        _NC_CACHE[key] = _build_nc(tuple(gps_jj), act_relu_groups)
    return _NC_CACHE[key]


def kernel(x, mask, Wv, bv, Wk, bk, Wq, bq, Wo, bo, _trace=False,
           _gps_jj=(0, 4), _act_relu_groups=4, **_ignored):
    x = np.asarray(x, dtype=np.float32)
    mask = np.asarray(mask, dtype=np.float32)
    bf = ml_dtypes.bfloat16
    weights = {
        "WqT": np.ascontiguousarray(np.asarray(Wq, np.float32).T.astype(bf)),
        "WkT": np.ascontiguousarray(np.asarray(Wk, np.float32).T.astype(bf)),
        "WvT": np.ascontiguousarray(np.asarray(Wv, np.float32).T.astype(bf)),
        "WoT": np.ascontiguousarray(np.asarray(Wo, np.float32).T.astype(bf)),
        "bq": np.ascontiguousarray(np.asarray(bq, np.float32)),
        "bk": np.ascontiguousarray(np.asarray(bk, np.float32)),
        "bv4": np.ascontiguousarray(
            np.tile(np.asarray(bv, np.float32), 4).astype(bf)
        ),
        "bo": np.ascontiguousarray(np.asarray(bo, np.float32)),
    }

    nc = _get_nc(_gps_jj, _act_relu_groups)
    in_maps = []
    for c in range(CORES):
        b, half = divmod(c, 2)
        xb = x[b]
        in_maps.append(
            dict(
                weights,
                xTq=np.ascontiguousarray(
                    xb[half * NQ:(half + 1) * NQ].T.astype(bf)
                ),
                xTkv=np.ascontiguousarray(xb.T.astype(bf)),
                maskT=np.ascontiguousarray(
                    mask[b, half * NQ:(half + 1) * NQ].T.astype(bf)
                ),
            )
        )
    res = run_bass_kernel_spmd(
        nc, in_maps, core_ids=list(range(CORES)), trace=_trace
    )
    out = np.empty((B, N, DOUT), dtype=np.float32)
    for c in range(CORES):
        b, half = divmod(c, 2)
        out[b, half * NQ:(half + 1) * NQ] = res.results[c]["y"]
    if _trace:
        return out, res
    return out
